# revision 5
# baseline (speedup 1.0000x reference)
"""DTAT sparse-attention transformer block kernel for 8 TRN2 NeuronCores.

Sharding: data-parallel over batch (2) x tensor-parallel over heads (4 per
core). Each core computes q/k/v projections for its 4 heads, the chunked
top-32-of-64 gated attention, and a partial output projection; the host sums
the 4 tensor-parallel partials per batch and adds bo.
"""
import math
import sys

sys.path.insert(0, "/opt/trn_rl_repo")

import numpy as np
import orjson

import concourse.bass as bass
import concourse.mybir as mybir
from concourse.bass_utils import run_bass_kernel_spmd
from concourse.tile import TileContext

F32 = mybir.dt.float32
AF = mybir.ActivationFunctionType
ALU = mybir.AluOpType

B, T, C, H = 2, 2048, 2048, 16
D = C // H            # 128
CS = 64               # chunk size
N = T // CS           # 32 chunks
TOPK = 32
HPC = 4               # heads per core
FW = HPC * D          # 512 per-core feature width
NEG = -1.0e9
P = 128
NT = T // P           # 16 token tiles
NQP = NT              # q chunk-pairs per head (128 tokens = 2 chunks)
NCC = C // P          # 16 contraction chunks


# --- workaround: this walrus build rejects >1 sync wait per instruction ----
def _split_multiwait(d):
    ctr = 0
    for f in d.get("functions", []):
        for bb in f.get("blocks", []):
            insts = bb.get("instructions", [])
            if not any(len(((i.get("sync_info") or {}).get("on_wait") or [])) > 1 for i in insts):
                continue
            new = []
            for inst in insts:
                si = inst.get("sync_info")
                ws = (si or {}).get("on_wait") or []
                if len(ws) > 1:
                    for w in ws[:-1]:
                        ctr += 1
                        new.append({
                            "debug": inst.get("debug", 0),
                            "engine": inst["engine"],
                            "ins": [], "outs": [],
                            "name": f"I-wsplit-{ctr}",
                            "opcode": "NoOp",
                            "sync_info": {"on_update": [], "on_wait": [w]},
                        })
                    si["on_wait"] = [ws[-1]]
                new.append(inst)
            bb["instructions"] = new
    return d


_orig_to_json_bytes = bass.Bass.to_json_bytes


def _patched_to_json_bytes(self):
    return orjson.dumps(_split_multiwait(orjson.loads(_orig_to_json_bytes(self))))


bass.Bass.to_json_bytes = _patched_to_json_bytes


def build_program():
    nc = bass.Bass()

    x_in = nc.declare_dram_parameter("x", [T, C], F32, isOutput=False)
    wq_in = nc.declare_dram_parameter("wq", [C, FW], F32, isOutput=False)
    wk_in = nc.declare_dram_parameter("wk", [C, FW], F32, isOutput=False)
    wv_in = nc.declare_dram_parameter("wv", [C, FW], F32, isOutput=False)
    wo_in = nc.declare_dram_parameter("wo", [FW, C], F32, isOutput=False)
    bq_in = nc.declare_dram_parameter("bq", [1, FW], F32, isOutput=False)
    bk_in = nc.declare_dram_parameter("bk", [1, FW], F32, isOutput=False)
    bv_in = nc.declare_dram_parameter("bv", [1, FW], F32, isOutput=False)
    gates_in = nc.declare_dram_parameter("gates", [P, HPC * NQP], F32, isOutput=False)
    ident_in = nc.declare_dram_parameter("ident", [P, P], F32, isOutput=False)
    ones_in = nc.declare_dram_parameter("ones", [1, 512], F32, isOutput=False)
    out_dram = nc.declare_dram_parameter("out", [T, C], F32, isOutput=True)

    with TileContext(nc) as tc:
        with (
            tc.tile_pool(name="const", bufs=1) as cpool,
            tc.tile_pool(name="at", bufs=1) as atpool,
            tc.tile_pool(name="spill", bufs=1, space="DRAM") as dpool,
        ):
            # DRAM spill for projection results (pool tiles so deps are tracked)
            qT_sp = dpool.tile([FW, T], F32, tag="qT_sp")
            kT_sp = dpool.tile([FW, T], F32, tag="kT_sp")
            v_sp = dpool.tile([T, FW], F32, tag="v_sp")
            ident = cpool.tile([P, P], F32)
            nc.sync.dma_start(out=ident[:], in_=ident_in[:])
            ones = cpool.tile([1, 512], F32)
            nc.sync.dma_start(out=ones[:], in_=ones_in[:])
            gates = cpool.tile([P, HPC * NQP], F32)
            nc.sync.dma_start(out=gates[:], in_=gates_in[:])
            brows = {}
            for nm, src in (("q", bq_in), ("k", bk_in), ("v", bv_in)):
                t = cpool.tile([1, FW], F32, tag=f"b{nm}", name=f"b{nm}")
                nc.sync.dma_start(out=t[:], in_=src[:])
                brows[nm] = t

            # A^T accumulators, one per head: [d=128, c=T]
            AT = [atpool.tile([P, T], F32, tag=f"AT{h}", name=f"AT{h}") for h in range(HPC)]

            # ---------------- Stage A: x^T build + q/k/v projections --------
            with (
                tc.tile_pool(name="sA", bufs=5) as sA,
                tc.tile_pool(name="xTp", bufs=2) as xTp,
                tc.tile_pool(name="wA", bufs=4) as wA,
                tc.tile_pool(name="evA", bufs=3) as evA,
                tc.tile_pool(name="psA", bufs=3, space="PSUM") as psA,
            ):
                for tp in range(4):  # panels of 512 tokens
                    xrow = []
                    for tt in range(4):
                        xr = sA.tile([P, C], F32, tag="xrow", name="xrow")
                        nc.sync.dma_start(out=xr[:], in_=x_in[tp * 512 + tt * P: tp * 512 + (tt + 1) * P, :])
                        xrow.append(xr)
                    xT = xTp.tile([P, NCC, 512], F32, tag="xT")  # [c-part, cc, t]
                    for cc in range(NCC):
                        bank = psA.tile([P, 512], F32, tag="trbank")
                        for tt in range(4):
                            nc.tensor.transpose(bank[:, tt * P:(tt + 1) * P], xrow[tt][:, cc * P:(cc + 1) * P], ident[:])
                        nc.scalar.activation(xT[:, cc, :], bank[:], AF.Copy)

                    # q^T / k^T: [f, t] orientation
                    for nm, w_in, spill in (("q", wq_in, qT_sp), ("k", wk_in, kT_sp)):
                        for ft in range(4):
                            bank = psA.tile([P, 512], F32, tag="pbank")
                            for cc in range(NCC):
                                w = wA.tile([P, P], F32, tag="wtile")
                                nc.sync.dma_start(out=w[:], in_=w_in[cc * P:(cc + 1) * P, ft * P:(ft + 1) * P])
                                nc.tensor.matmul(bank[:], w[:], xT[:, cc, :], start=(cc == 0), stop=False)
                            nc.tensor.matmul(bank[:], brows[nm][:, ft * P:(ft + 1) * P], ones[:], start=False, stop=True)
                            ev = evA.tile([P, 512], F32, tag="evac")
                            nc.scalar.activation(ev[:], bank[:], AF.Copy)
                            nc.sync.dma_start(out=spill[ft * P:(ft + 1) * P, tp * 512:(tp + 1) * 512], in_=ev[:])

                    # v: [t, f] orientation
                    for tt in range(4):
                        bank = psA.tile([P, 512], F32, tag="pbank")
                        for cc in range(NCC):
                            wv = wA.tile([P, 512], F32, tag="wvtile")
                            nc.sync.dma_start(out=wv[:], in_=wv_in[cc * P:(cc + 1) * P, :])
                            nc.tensor.matmul(bank[:], xT[:, cc, tt * P:(tt + 1) * P], wv[:], start=(cc == 0), stop=False)
                        # bias: out[t, f] += 1 * bv[f]
                        nc.tensor.matmul(bank[:], ones[:, 0:P], brows["v"][:], start=False, stop=True)
                        ev = evA.tile([P, 512], F32, tag="evac")
                        nc.scalar.activation(ev[:], bank[:], AF.Copy)
                        nc.sync.dma_start(out=v_sp[tp * 512 + tt * P: tp * 512 + (tt + 1) * P, :], in_=ev[:])

            # ---------------- Stage B: attention per head -------------------
            with (
                tc.tile_pool(name="hB", bufs=2) as hB,
                tc.tile_pool(name="sB", bufs=2) as sB,
                tc.tile_pool(name="m8B", bufs=6) as m8B,
                tc.tile_pool(name="zB", bufs=3) as zB,
                tc.tile_pool(name="ptB", bufs=5) as ptB,
                tc.tile_pool(name="psS", bufs=3, space="PSUM") as psS,
                tc.tile_pool(name="psT", bufs=2, space="PSUM") as psT,
                tc.tile_pool(name="psAV", bufs=2, space="PSUM") as psAV,
            ):
                for h in range(HPC):
                    khT = hB.tile([P, T], F32, tag="khT")
                    nc.sync.dma_start(out=khT[:], in_=kT_sp[h * P:(h + 1) * P, :])
                    qhT = hB.tile([P, T], F32, tag="qhT")
                    nc.sync.dma_start(out=qhT[:], in_=qT_sp[h * P:(h + 1) * P, :])
                    Vh = hB.tile([P, NT, P], F32, tag="Vh")
                    for et in range(NT):
                        nc.sync.dma_start(out=Vh[:, et, :], in_=v_sp[et * P:(et + 1) * P, h * P:(h + 1) * P])

                    for qp in range(NQP):
                        gcol = gates[:, h * NQP + qp: h * NQP + qp + 1]
                        St = sB.tile([P, T], F32, tag="St")
                        for eb in range(4):
                            bank = psS.tile([P, 512], F32, tag="sbank")
                            nc.tensor.matmul(bank[:], qhT[:, qp * P:(qp + 1) * P], khT[:, eb * 512:(eb + 1) * 512],
                                             start=True, stop=True)
                            nc.scalar.activation(St[:, eb * 512:(eb + 1) * 512], bank[:], AF.Copy, scale=gcol)

                        # top-32-of-64 selection per kv chunk
                        zap = sB.tile([P, T], F32, tag="zap")
                        for g32 in range(N):
                            sl = slice(g32 * CS, (g32 + 1) * CS)
                            m8 = m8B.tile([P, 8], F32, tag="m8")
                            nc.vector.max(out=m8[:], in_=St[:, sl])
                            nc.vector.match_replace(out=zap[:, sl], in_to_replace=m8[:], in_values=St[:, sl], imm_value=NEG)
                            for r in range(3):
                                m8b = m8B.tile([P, 8], F32, tag="m8")
                                nc.vector.max(out=m8b[:], in_=zap[:, sl])
                                nc.vector.match_replace(out=zap[:, sl], in_to_replace=m8b[:], in_values=zap[:, sl], imm_value=NEG)

                        # keep top-32 values, zero the rest; then exp in place
                        sp_ = sB.tile([P, T], F32, tag="sp")
                        nc.vector.scalar_tensor_tensor(out=sp_[:], in0=zap[:], scalar=-1.0e8, in1=St[:],
                                                       op0=ALU.is_le, op1=ALU.mult)
                        nc.scalar.activation(sp_[:], sp_[:], AF.Exp)
                        p3 = sp_[:].rearrange("p (g e) -> p g e", g=N)

                        # per-chunk sums -> Z, reciprocal, normalize in place
                        scr = zB.tile([P, N, CS // 2], F32, tag="scr")
                        nc.gpsimd.tensor_tensor(out=scr[:], in0=p3[:, :, 0:32], in1=p3[:, :, 32:64], op=ALU.add)
                        w = 16
                        while w >= 1:
                            nc.gpsimd.tensor_tensor(out=scr[:, :, 0:w], in0=scr[:, :, 0:w], in1=scr[:, :, w:2 * w], op=ALU.add)
                            w //= 2
                        zt = zB.tile([P, N], F32, tag="zt")
                        nc.vector.tensor_copy(zt[:], scr[:, :, 0:1].rearrange("p g e -> p (g e)"))
                        rz = zB.tile([P, N], F32, tag="rz")
                        nc.vector.reciprocal(rz[:], zt[:])
                        rzb = rz[:].rearrange("p (g e) -> p g e", g=N).to_broadcast([P, N, CS])
                        nc.gpsimd.tensor_tensor(out=p3, in0=p3, in1=rzb, op=ALU.mult)

                        # transpose p-hat blocks and accumulate PV into A^T
                        avbank = psAV.tile([P, P], F32, tag="avbank")
                        for mq in range(4):
                            ptbank = psT.tile([P, 512], F32, tag="ptbank")
                            for j in range(4):
                                mb = mq * 4 + j
                                nc.tensor.transpose(ptbank[:, j * P:(j + 1) * P], sp_[:, mb * P:(mb + 1) * P], ident[:])
                            ptsb = ptB.tile([P, 512], F32, tag="ptsb")
                            nc.scalar.activation(ptsb[:], ptbank[:], AF.Copy)
                            for j in range(4):
                                mb = mq * 4 + j
                                nc.tensor.matmul(avbank[:], Vh[:, mb, :], ptsb[:, j * P:(j + 1) * P],
                                                 start=(mb == 0), stop=(mb == 15))
                        nc.scalar.activation(AT[h][:, qp * P:(qp + 1) * P], avbank[:], AF.Copy)

            # ---------------- Stage C: output projection --------------------
            with (
                tc.tile_pool(name="woC", bufs=1) as woC,
                tc.tile_pool(name="oC", bufs=3) as oC,
                tc.tile_pool(name="psO", bufs=3, space="PSUM") as psO,
            ):
                wo_res = [woC.tile([P, C], F32, tag=f"wo{fc}", name=f"wo{fc}") for fc in range(HPC)]
                for fc in range(HPC):
                    nc.sync.dma_start(out=wo_res[fc][:], in_=wo_in[fc * P:(fc + 1) * P, :])
                for tt in range(NT):
                    for cb in range(4):
                        bank = psO.tile([P, 512], F32, tag="obank")
                        for fc in range(HPC):
                            nc.tensor.matmul(bank[:], AT[fc][:, tt * P:(tt + 1) * P],
                                             wo_res[fc][:, cb * 512:(cb + 1) * 512],
                                             start=(fc == 0), stop=(fc == HPC - 1))
                        osb = oC.tile([P, 512], F32, tag="osb")
                        nc.scalar.activation(osb[:], bank[:], AF.Copy)
                        nc.sync.dma_start(out=out_dram[tt * P:(tt + 1) * P, cb * 512:(cb + 1) * 512], in_=osb[:])

    return nc


_NC_CACHE = None


def _sigmoid(v):
    return 1.0 / (1.0 + np.exp(-v.astype(np.float64)))


def kernel(x, importance_scores, temperatures, Wq, bq, Wk, bk, Wv, bv, Wo, bo):
    global _NC_CACHE
    x = np.asarray(x, dtype=np.float32)
    importance_scores = np.asarray(importance_scores, dtype=np.float32)
    temperatures = np.asarray(temperatures, dtype=np.float32)
    Wq, bq = np.asarray(Wq, np.float32), np.asarray(bq, np.float32)
    Wk, bk = np.asarray(Wk, np.float32), np.asarray(bk, np.float32)
    Wv, bv = np.asarray(Wv, np.float32), np.asarray(bv, np.float32)
    Wo, bo = np.asarray(Wo, np.float32), np.asarray(bo, np.float32)

    if _NC_CACHE is None:
        _NC_CACHE = build_program()
    nc = _NC_CACHE

    scale = 1.0 / math.sqrt(D)
    temp = np.clip(temperatures, 0.1, 100.0)

    ident = np.eye(P, dtype=np.float32)
    ones = np.ones((1, 512), np.float32)

    in_maps = []
    for core in range(8):
        b = core // 4
        h0 = (core % 4) * HPC
        fsl = slice(h0 * D, (h0 + HPC) * D)
        # gates[c, h*16+qp] = sigmoid((sigmoid(imp)-0.5)*10) * scale / temp
        g = np.empty((P, HPC * NQP), np.float32)
        for hh in range(HPC):
            imp = importance_scores[b, :, h0 + hh]
            mw = _sigmoid((_sigmoid(imp) - 0.5) * 10.0) * scale / temp[b, h0 + hh]
            g[:, hh * NQP:(hh + 1) * NQP] = mw.reshape(NQP, P).T.astype(np.float32)
        in_maps.append({
            "x": np.ascontiguousarray(x[b]),
            "wq": np.ascontiguousarray(Wq[:, fsl]),
            "wk": np.ascontiguousarray(Wk[:, fsl]),
            "wv": np.ascontiguousarray(Wv[:, fsl]),
            "wo": np.ascontiguousarray(Wo[fsl, :]),
            "bq": np.ascontiguousarray(bq[fsl]).reshape(1, FW),
            "bk": np.ascontiguousarray(bk[fsl]).reshape(1, FW),
            "bv": np.ascontiguousarray(bv[fsl]).reshape(1, FW),
            "gates": g,
            "ident": ident,
            "ones": ones,
        })

    res = run_bass_kernel_spmd(nc, in_maps, list(range(8)))
    kernel.last_exec_time_ns = res.exec_time_ns

    out = np.empty((B, T, C), np.float32)
    for b in range(B):
        acc = res.results[b * 4]["out"].astype(np.float32).copy()
        for i in range(1, 4):
            acc += res.results[b * 4 + i]["out"]
        # normalizer is exactly N (=32) chunks each summing to 1; folded here
        out[b] = acc / np.float32(N) + bo
    return out
